# revision 26
# baseline (speedup 1.0000x reference)
"""MoE (8 experts, top-2) Trainium2 kernel, expert-parallel across 8 NeuronCores.

v2 strategy (each core owns one expert, gate replicated):
  - Router logits in exact fp32 via bf16 hi/lo splitting, with [gw_hi|gw_lo]
    stacked into one 16-wide lhsT so terms (hi@xh + lo@xh) share one rhs
    stream; third term hi@xl streams x_lo. 2 rhs streams instead of 3.
  - Top-2 + renormalized gate weights on DVE (as before).
  - Compaction WITHOUT gpsimd sparse_gather: per-token slot ids via PE
    prefix-sum matmuls (strict-triangular ones), then the compacted token
    list is produced directly in the swdge 16-wrapped [16, cap/16] layout by
    an accumulating matmul with data-dependent one-hot operands:
      idx[m, f] = sum_p onehot(slot%16 == m)[p] * (tokid * onehot(slot//16 == f))[p]
    This also yields the per-slot coef in the same pass.
  - Dispatch via gpsimd dma_gather(transpose=True): gathers the selected x
    rows from DRAM AND transposes them into [128, H/128, slot] layout in one
    swdge op, split across 2 hardware queues. No PE transposes, no index
    relayout round trips.
  - Weights are host-repacked so every per-slice DMA is a dense 2KB-per-
    partition transfer (128 descriptors instead of 1024).
  - Expert MLP in bf16: (silu(x@w1) * (x@w3)) @ w2, scaled by gate coef.
  - Host combines: out[idx] += yT.T[:cnt].
"""
import sys

sys.path.insert(0, "/opt/trn_rl_repo")

import numpy as np

T, H, II, E = 2048, 1024, 4096, 8
P = 128
NT = T // P          # 16 token tiles
HC = H // P          # 8 hidden chunks
IC = II // P         # 32 intermediate chunks
NCORES = 8

_build_cache = {}


def _build(cap):
    """Build + schedule the per-core Tile kernel for token capacity `cap`."""
    import concourse.bass as bass
    import concourse.bacc as bacc
    import concourse.mybir as mybir
    from concourse.tile import TileContext

    f32 = mybir.dt.float32
    i32 = mybir.dt.int32
    i16 = mybir.dt.int16
    u32 = mybir.dt.uint32
    bf16 = mybir.dt.bfloat16
    fp16 = mybir.dt.float16
    AF = mybir.ActivationFunctionType
    OP = mybir.AluOpType

    assert cap in (560, 1024, 2048), cap
    cf = cap // 16       # wrapped free cols
    UNSEL = 32 * cf      # slot for unselected tokens (sdiv out of range)
    # gather splits: (num_idxs %128==0, real slot cols used by the MLP)
    _gspec = {560: [(256, 256), (384, 304)],
              1024: [(512, 512), (512, 512)],
              2048: [(512, 512)] * 4}[cap]
    cfp = sum(g[0] for g in _gspec) // 16   # padded idx cols

    nc = bacc.Bacc("TRN2", target_bir_lowering=False, num_swdge_queues=2)

    # ---- I/O ----
    xth = nc.declare_dram_parameter("xth", [H, T], bf16, isOutput=False)
    xtl = nc.declare_dram_parameter("xtl", [H, T], bf16, isOutput=False)
    GWS = 40  # [gw_hi (8) | zeros (24) | gw_lo (8)] -- lo lands at psum partitions 32:40
    gws = nc.declare_dram_parameter("gws", [P, HC * GWS], bf16, isOutput=False)
    w1 = nc.declare_dram_parameter("w1", [P, IC * H], bf16, isOutput=False)
    w3 = nc.declare_dram_parameter("w3", [P, IC * H], bf16, isOutput=False)
    w2 = nc.declare_dram_parameter("w2", [P, HC * II], bf16, isOutput=False)
    oh = nc.declare_dram_parameter("oh", [P, NT * E], f32, isOutput=False)
    tokid = nc.declare_dram_parameter("tokid", [P, NT], f32, isOutput=False)
    tokh_d = nc.declare_dram_parameter("tokh", [P, NT], fp16, isOutput=False)
    x_d = nc.declare_dram_parameter("x", [T, H], bf16, isOutput=False)
    io16_d = nc.declare_dram_parameter("io16", [P, NT * 16], fp16, isOutput=False)
    iocf_d = nc.declare_dram_parameter("iocf", [P, NT * cf], fp16, isOutput=False)
    vt16_d = nc.declare_dram_parameter("vt16", [16, P], fp16, isOutput=False)
    stril_d = nc.declare_dram_parameter("stril", [P, P], bf16, isOutput=False)
    ones128_d = nc.declare_dram_parameter("ones128", [P, 1], bf16, isOutput=False)
    ident = nc.declare_dram_parameter("ident", [P, P], f32, isOutput=False)

    o_yt = nc.declare_dram_parameter("o_yt", [H, cap], f32, isOutput=True)
    o_idx = nc.declare_dram_parameter("o_idx", [cap], i32, isOutput=True)
    o_cnt = nc.declare_dram_parameter("o_cnt", [1, 1], u32, isOutput=True)

    with TileContext(nc) as tc:
        with (
            tc.tile_pool(name="sb", bufs=1) as sb,
            tc.tile_pool(name="sbw", bufs=2) as sbw,
            tc.tile_pool(name="psum", bufs=2, space="PSUM") as psg,
            tc.tile_pool(name="drp", bufs=1, space="DRAM") as drp,
        ):
            d_cf = drp.tile([cap], f32, tag="d_cf")

            # stacked gate weights [128, (hc, 40)], host-packed dense
            gw_s = sb.tile([P, HC * GWS], bf16, tag="gws")
            nc.sync.dma_start(out=gw_s[:], in_=gws[:])

            warm_n = [0]

            def warm(dep_ap, kdim):
                n = min(dep_ap.shape[-1] if len(dep_ap.shape) == 2 else dep_ap.free_size(), 512)
                wps = psg.tile([8, n], f32, tag="mm1", name=f"warm{warm_n[0]}")
                warm_n[0] += 1
                nc.tensor.matmul(
                    out=wps[:],
                    lhsT=idt[0:kdim, 0:8],
                    rhs=dep_ap,
                    start=True, stop=True,
                )

            # ---- A. router: 2 rhs streams ----
            # ps_a[ng] [16,512] accumulates [gwh|gwl]^T @ x_hi
            # ps_b[ng] [8,512]  accumulates gwh^T @ x_lo
            # one PSUM bank per group; BOTH x_hi and x_lo streams accumulate
            # into it with the same [gwh|pad|gwl] lhsT:
            #   rows 0:8  = gwh@x_hi + gwh@x_lo
            #   rows 32:40 = gwl@x_hi (+ gwl@x_lo, negligible & harmless)
            ps_ab = [psg.tile([40, 512], f32, tag=f"psab{ng}", name=f"psab{ng}", bufs=1)
                     for ng in range(4)]
            for hc in range(HC):
                xt_h = sbw.tile([P, T], bf16, tag="xth", bufs=8)
                xt_l = sbw.tile([P, T], bf16, tag="xtl", bufs=8)
                if hc == 0:
                    # first chunk in halves, hi+lo on both queues, so the
                    # first router matmuls start as early as possible
                    nc.sync.dma_start(out=xt_h[:, 0:1024], in_=xth[0:P, 0:1024])
                    nc.gpsimd.dma_start(out=xt_l[:, 0:1024], in_=xtl[0:P, 0:1024])
                    nc.sync.dma_start(out=xt_h[:, 1024:T], in_=xth[0:P, 1024:T])
                    nc.gpsimd.dma_start(out=xt_l[:, 1024:T], in_=xtl[0:P, 1024:T])
                else:
                    nc.sync.dma_start(out=xt_h[:], in_=xth[hc * P:(hc + 1) * P, :])
                    nc.gpsimd.dma_start(out=xt_l[:], in_=xtl[hc * P:(hc + 1) * P, :])
                for ng in range(4):
                    nc.tensor.matmul(
                        out=ps_ab[ng][:],
                        lhsT=gw_s[:, hc * GWS:(hc + 1) * GWS],
                        rhs=xt_h[:, ng * 512:(ng + 1) * 512],
                        start=(hc == 0), stop=False,
                    )
                for ng in range(4):
                    nc.tensor.matmul(
                        out=ps_ab[ng][:],
                        lhsT=gw_s[:, hc * GWS:(hc + 1) * GWS],
                        rhs=xt_l[:, ng * 512:(ng + 1) * 512],
                        start=False, stop=(hc == HC - 1),
                    )
                # keep the PE p-state ramped while waiting on the x stream
                for wb in range(3):
                    wpsr = psg.tile([8, 512], f32, tag="mm0", name=f"wr{hc}_{wb}")
                    nc.tensor.matmul(
                        out=wpsr[:],
                        lhsT=gw_s[:, 0:8],
                        rhs=xt_h[:, wb * 512:(wb + 1) * 512],
                        start=True, stop=True,
                    )


            # ---- constants (loaded behind the x stream) ----
            idt = sb.tile([P, P], f32, tag="idt")
            nc.scalar.dma_start(out=idt[:], in_=ident[:])
            oh_sb = sb.tile([P, NT * E], f32, tag="oh")
            nc.scalar.dma_start(out=oh_sb[:], in_=oh[:])
            tk = sb.tile([P, NT], f32, tag="tk")
            nc.scalar.dma_start(out=tk[:], in_=tokid[:])
            tokh = sb.tile([P, NT], fp16, tag="tokh")
            nc.scalar.dma_start(out=tokh[:], in_=tokh_d[:])
            io16 = sb.tile([P, NT * 16], fp16, tag="io16")
            nc.scalar.dma_start(out=io16[:], in_=io16_d[:])
            iocf = sb.tile([P, NT * cf], fp16, tag="iocf")
            nc.scalar.dma_start(out=iocf[:], in_=iocf_d[:])
            vt16 = sb.tile([16, P], fp16, tag="vt16")
            nc.scalar.dma_start(out=vt16[:], in_=vt16_d[:])
            stril = sb.tile([P, P], bf16, tag="stril")
            nc.scalar.dma_start(out=stril[:], in_=stril_d[:])
            ones128 = sb.tile([P, 1], bf16, tag="ones128")
            nc.scalar.dma_start(out=ones128[:], in_=ones128_d[:])
            onesP = sb.tile([1, P], f32, tag="onesP")
            nc.vector.memset(onesP[:], 1.0)

            # fold: logitsT [8, T] = ps_a[0:8] + ps_a[8:16] + ps_b
            # (one PSUM operand per DVE op)
            logitsT = sb.tile([E, T], f32, tag="logitsT")
            for ng in range(4):
                gsl = slice(ng * 512, (ng + 1) * 512)
                nc.scalar.activation(
                    out=logitsT[:, gsl], in_=ps_ab[ng][0:E, :], func=AF.Copy,
                )
                nc.vector.tensor_add(
                    out=logitsT[:, gsl],
                    in0=logitsT[:, gsl], in1=ps_ab[ng][32:32 + E, :],
                )

            # ---- B. transpose logitsT -> l_all [128, (16, 8)] ----
            l_all = sb.tile([P, NT * E], f32, tag="l_all")
            for ci in range(NT):
                tp = psg.tile([P, E], f32, tag="mm0")
                nc.tensor.transpose(
                    out=tp[:],
                    in_=logitsT[:, ci * P:(ci + 1) * P],
                    identity=idt[0:E, 0:E],
                )
                nc.vector.tensor_copy(out=l_all[:, ci * E:(ci + 1) * E], in_=tp[:])

            # ---- C. top-2 + coef ----
            l3 = l_all[:].rearrange("p (t e) -> p t e", e=E)
            m1 = sb.tile([P, NT, 1], f32, tag="m1")
            nc.vector.reduce_max(out=m1[:], in_=l3[:], axis=mybir.AxisListType.X)
            eqm = sb.tile([P, NT, E], f32, tag="eqm")
            nc.vector.tensor_tensor(
                out=eqm[:], in0=l3[:], in1=m1[:].to_broadcast([P, NT, E]),
                op=OP.is_equal,
            )
            l3m = sb.tile([P, NT, E], f32, tag="l3m")
            nc.vector.tensor_scalar(l3m[:], eqm[:], -1e30, None, op0=OP.mult)
            nc.vector.tensor_add(out=l3m[:], in0=l3m[:], in1=l3[:])
            m2 = sb.tile([P, NT, 1], f32, tag="m2")
            nc.vector.reduce_max(out=m2[:], in_=l3m[:], axis=mybir.AxisListType.X)

            warm(m1[:, :, 0], P)
            dq = sb.tile([P, NT], f32, tag="dq")
            nc.vector.tensor_sub(out=dq[:], in0=m2[:, :, 0], in1=m1[:, :, 0])
            q = sb.tile([P, NT], f32, tag="q")
            nc.scalar.activation(out=q[:], in_=dq[:], func=AF.Exp)
            s = sb.tile([P, NT], f32, tag="s")
            nc.vector.tensor_scalar_add(s[:], q[:], 1.0)
            wt1 = sb.tile([P, NT], f32, tag="wt1")
            nc.vector.reciprocal(wt1[:], s[:])
            wt2 = sb.tile([P, NT], f32, tag="wt2")
            nc.vector.tensor_mul(out=wt2[:], in0=q[:], in1=wt1[:])

            le_m = sb.tile([P, NT, E], f32, tag="lem")
            nc.vector.tensor_mul(
                out=le_m[:], in0=l3[:], in1=oh_sb[:].rearrange("p (t e) -> p t e", e=E)
            )
            le = sb.tile([P, NT], f32, tag="le")
            nc.vector.reduce_sum(
                out=le[:].rearrange("p (t o) -> p t o", o=1),
                in_=le_m[:],
                axis=mybir.AxisListType.X,
            )

            eq1 = sb.tile([P, NT], f32, tag="eq1")
            eq2 = sb.tile([P, NT], f32, tag="eq2")
            nc.vector.tensor_tensor(out=eq1[:], in0=le[:], in1=m1[:, :, 0], op=OP.is_equal)
            nc.vector.tensor_tensor(out=eq2[:], in0=le[:], in1=m2[:, :, 0], op=OP.is_equal)
            coef = sb.tile([P, NT], f32, tag="coef")
            t1 = sb.tile([P, NT], f32, tag="t1")
            nc.vector.tensor_mul(out=coef[:], in0=eq1[:], in1=wt1[:])
            nc.vector.tensor_mul(out=t1[:], in0=eq2[:], in1=wt2[:])
            nc.vector.tensor_add(out=coef[:], in0=coef[:], in1=t1[:])
            selm = sb.tile([P, NT], f32, tag="selm")
            nc.vector.tensor_add(out=selm[:], in0=eq1[:], in1=eq2[:])
            warm(coef[:], P)

            # ---- D. slot assignment (column-major scan order) ----
            selm_bf = sb.tile([P, NT], bf16, tag="selmbf")
            nc.vector.tensor_copy(out=selm_bf[:], in_=selm[:])
            ps_pref = psg.tile([P, NT], f32, tag="mm0", name="pspref")
            nc.tensor.matmul(out=ps_pref[:], lhsT=stril[:], rhs=selm_bf[:], start=True, stop=True)
            ps_cs = psg.tile([1, NT], f32, tag="mm1", name="pscs")
            nc.tensor.matmul(out=ps_cs[:], lhsT=ones128[:], rhs=selm_bf[:], start=True, stop=True)
            # exclusive cumsum of column counts on [1,16]
            csA = sb.tile([1, NT], f32, tag="csA")
            nc.vector.tensor_copy(out=csA[:], in_=ps_cs[:])
            csB = sb.tile([1, NT], f32, tag="csB")
            cur = csA
            oth = csB
            for sh in (1, 2, 4, 8):
                nc.vector.tensor_copy(out=oth[:, 0:sh], in_=cur[:, 0:sh])
                nc.vector.tensor_add(out=oth[:, sh:NT], in0=cur[:, sh:NT], in1=cur[:, 0:NT - sh])
                cur, oth = oth, cur
            # cur = inclusive cumsum; cnt = cur[0, NT-1]
            cnt_u = sb.tile([1, 1], u32, tag="cntu")
            nc.vector.tensor_copy(out=cnt_u[:], in_=cur[:, NT - 1:NT])
            nc.scalar.dma_start(out=o_cnt[:], in_=cnt_u[:])
            o_excl = sb.tile([1, NT], f32, tag="oexcl")
            nc.vector.tensor_sub(out=o_excl[:], in0=cur[:], in1=ps_cs[:])
            ps_orep = psg.tile([P, NT], f32, tag="mm0", name="psorep")
            nc.tensor.matmul(out=ps_orep[:], lhsT=onesP[:], rhs=o_excl[:], start=True, stop=True)
            orep_sb = sb.tile([P, NT], f32, tag="orep")
            nc.vector.tensor_copy(out=orep_sb[:], in_=ps_orep[:])

            slot = sb.tile([P, NT], f32, tag="slot")
            nc.vector.tensor_add(out=slot[:], in0=ps_pref[:], in1=orep_sb[:])
            nc.vector.tensor_mul(out=slot[:], in0=slot[:], in1=selm[:])
            tsl = sb.tile([P, NT], f32, tag="tsl")
            nc.vector.tensor_scalar(tsl[:], selm[:], -float(UNSEL), float(UNSEL), op0=OP.mult, op1=OP.add)
            nc.vector.tensor_add(out=slot[:], in0=slot[:], in1=tsl[:])
            warm(slot[:], P)

            # sdiv = floor(slot/16) via round(slot/16 - bias); smod = slot - 16*sdiv
            sd0 = sb.tile([P, NT], f32, tag="sd0")
            nc.vector.tensor_scalar(sd0[:], slot[:], 1.0 / 16.0, -0.46875, op0=OP.mult, op1=OP.add)
            sdi = sb.tile([P, NT], i32, tag="sdi")
            nc.vector.tensor_copy(out=sdi[:], in_=sd0[:])
            sdivf = sb.tile([P, NT], f32, tag="sdivf")
            nc.vector.tensor_copy(out=sdivf[:], in_=sdi[:])
            smf = sb.tile([P, NT], f32, tag="smf")
            nc.vector.tensor_scalar(smf[:], sdivf[:], -16.0, 0.0, op0=OP.mult, op1=OP.add)
            nc.vector.tensor_add(out=smf[:], in0=smf[:], in1=slot[:])
            sdivh = sb.tile([P, NT], fp16, tag="sdivh")
            nc.vector.tensor_copy(out=sdivh[:], in_=sdivf[:])
            smodh = sb.tile([P, NT], fp16, tag="smodh")
            nc.vector.tensor_copy(out=smodh[:], in_=smf[:])
            coefh = sb.tile([P, NT], fp16, tag="coefh")
            nc.vector.tensor_copy(out=coefh[:], in_=coef[:])

            # one-hot masks, built in token-chunk halves so the inversion
            # matmuls of half 0 overlap the DVE work of half 1
            LH = sb.tile([P, NT * 16], fp16, tag="LH")
            fmask = sb.tile([P, NT * cf], fp16, tag="fmask")
            RHS = sb.tile([P, NT * 2 * cf], fp16, tag="RHS")
            LH3 = LH[:].rearrange("p (c m) -> p c m", m=16)
            io3 = io16[:].rearrange("p (c m) -> p c m", m=16)
            sm3 = smodh[:].rearrange("p (c o) -> p c o", o=1)
            fm3 = fmask[:].rearrange("p (c f) -> p c f", f=cf)
            ioc3 = iocf[:].rearrange("p (c f) -> p c f", f=cf)
            sd3 = sdivh[:].rearrange("p (c o) -> p c o", o=1)
            rhs4 = RHS[:].rearrange("p (c z f) -> p c z f", z=2, f=cf)
            fm4 = fmask[:].rearrange("p (c o f) -> p c o f", o=1, f=cf)
            tk4 = tokh[:].rearrange("p (c o w) -> p c o w", o=1, w=1)
            ch4 = coefh[:].rearrange("p (c o w) -> p c o w", o=1, w=1)
            HB = NT // 2
            for h in (0, 1):
                cs = slice(h * HB, (h + 1) * HB)
                nc.vector.tensor_tensor(
                    out=fm3[:, cs, :], in0=ioc3[:, cs, :],
                    in1=sd3[:, cs, :].to_broadcast([P, HB, cf]),
                    op=OP.is_equal,
                )
                nc.vector.tensor_tensor(
                    out=rhs4[:, cs, 0:1, :], in0=fm4[:, cs, :, :],
                    in1=tk4[:, cs, :, :].to_broadcast([P, HB, 1, cf]),
                    op=OP.mult,
                )
                nc.vector.tensor_tensor(
                    out=rhs4[:, cs, 1:2, :], in0=fm4[:, cs, :, :],
                    in1=ch4[:, cs, :, :].to_broadcast([P, HB, 1, cf]),
                    op=OP.mult,
                )
                nc.vector.tensor_tensor(
                    out=LH3[:, cs, :], in0=io3[:, cs, :],
                    in1=sm3[:, cs, :].to_broadcast([P, HB, 16]),
                    op=OP.is_equal,
                )

            # inversion matmuls: inv [16, 2*cf] = sum_c LH_c^T @ RHS_c
            # inv[m, f]      = token id at slot f*16+m  (0 if unfilled)
            # inv[m, cf+f]   = gate coef of slot f*16+m (0 if unfilled)
            inv = psg.tile([16, 2 * cf], f32, tag="mm1", name="inv")
            for c in range(NT):
                nc.tensor.matmul(
                    out=inv[:],
                    lhsT=LH[:, c * 16:(c + 1) * 16],
                    rhs=RHS[:, c * 2 * cf:(c + 1) * 2 * cf],
                    start=(c == 0), stop=(c == NT - 1),
                )
            idxh = sb.tile([16, cf], fp16, tag="idxh")
            nc.vector.tensor_copy(out=idxh[:], in_=inv[:, 0:cf])
            coefw = sb.tile([16, cf], f32, tag="coefw")
            nc.vector.tensor_copy(out=coefw[:], in_=inv[:, cf:2 * cf])
            idx_i = sb.tile([16, cf], i32, tag="idxi")
            nc.vector.tensor_copy(out=idx_i[:], in_=inv[:, 0:cf])
            nc.scalar.dma_start(out=o_idx[:].rearrange("(f p) -> p f", p=16), in_=idx_i[:])

            # replicate wrapped idx to all 8 core groups -> int16 (pad cols = 0)
            ps_rep = psg.tile([P, cf], f32, tag="mm0", name="psrep")
            nc.tensor.matmul(out=ps_rep[:], lhsT=vt16[:], rhs=idxh[:], start=True, stop=True)
            idxs16 = sb.tile([P, cfp], i16, tag="idxs16")
            nc.vector.memset(idxs16[:], 0)
            nc.vector.tensor_copy(out=idxs16[:, 0:cf], in_=ps_rep[:])
            warm(slot[:], P)

            # ---- E. dispatch: swdge gather+transpose straight from DRAM x ----
            # xg[k][p, hc*nidx + s] = x[idx[goff+s], hc*128 + p]
            xg = []
            icol = 0
            for k, (nidx, _real) in enumerate(_gspec):
                g = sb.tile([P, HC * nidx], bf16, tag=f"xg{k}", name=f"xg{k}")
                nc.gpsimd.dma_gather(
                    out_ap=g[:].rearrange("p (q s) -> p q s", s=nidx),
                    in_ap=x_d[:],
                    idxs_ap=idxs16[:, icol:icol + nidx // 16],
                    num_idxs=nidx,
                    num_idxs_reg=nidx,
                    elem_size=H,
                    transpose=True,
                    single_packet=False,
                    queue_num=k % 2,
                )
                xg.append(g)
                icol += nidx // 16

            # ---- F. coef broadcast [128, cap] via dense DRAM round trip ----
            tpc = psg.tile([cf, 16], f32, tag="mm1", name="tpc")
            nc.tensor.transpose(out=tpc[:], in_=coefw[:], identity=idt[0:16, 0:16])
            ct = sb.tile([cf, 16], f32, tag="ct")
            nc.vector.tensor_copy(out=ct[:], in_=tpc[:])
            nc.scalar.dma_start(out=d_cf[:].rearrange("(f p) -> f p", p=16), in_=ct[:])
            vrow = sb.tile([1, cap], f32, tag="vrow")
            nc.scalar.dma_start(out=vrow[:], in_=d_cf[0:cap].rearrange("(o c) -> o c", o=1))
            cbc = sb.tile([P, cap], f32, tag="cbc")
            goff = 0
            for k, (nidx, real) in enumerate(_gspec):
                cb_ps = psg.tile([P, real], f32, tag="mm0", name=f"cb{k}")
                nc.tensor.matmul(
                    out=cb_ps[:], lhsT=onesP[:],
                    rhs=vrow[:, goff:goff + real], start=True, stop=True,
                )
                nc.vector.tensor_copy(out=cbc[:, goff:goff + real], in_=cb_ps[:])
                goff += real

            # ---- G. h1 = x@w1, h3 = x@w3 (transposed), fused silu*mul ----
            actT = [sb.tile([P, cap], bf16, tag=f"actT{ic}", name=f"actT{ic}") for ic in range(IC)]
            for ic in range(IC):
                w1_sl = sbw.tile([P, H], bf16, tag="w1sl", bufs=4)
                nc.sync.dma_start(out=w1_sl[:], in_=w1[:, ic * H:(ic + 1) * H])
                w3_sl = sbw.tile([P, H], bf16, tag="w3sl", bufs=4)
                nc.sync.dma_start(out=w3_sl[:], in_=w3[:, ic * H:(ic + 1) * H])
                goff = 0
                for k, (nidx, real) in enumerate(_gspec):
                    gs = slice(goff, goff + real)
                    ps1 = psg.tile([P, real], f32, tag="mm0", name=f"ps1_{ic}_{k}")
                    ps3 = psg.tile([P, real], f32, tag="mm1", name=f"ps3_{ic}_{k}")
                    for hc in range(HC):
                        nc.tensor.matmul(
                            out=ps1[:],
                            lhsT=w1_sl[:, hc * P:(hc + 1) * P],
                            rhs=xg[k][:, hc * nidx:hc * nidx + real],
                            start=(hc == 0), stop=(hc == HC - 1),
                        )
                    for hc in range(HC):
                        nc.tensor.matmul(
                            out=ps3[:],
                            lhsT=w3_sl[:, hc * P:(hc + 1) * P],
                            rhs=xg[k][:, hc * nidx:hc * nidx + real],
                            start=(hc == 0), stop=(hc == HC - 1),
                        )
                    sl = sbw.tile([P, real], f32, tag="silu", name=f"silu{ic}_{k}")
                    nc.scalar.activation(out=sl[:], in_=ps1[:], func=AF.Silu)
                    nc.vector.tensor_mul(out=actT[ic][:, gs], in0=sl[:], in1=ps3[:])
                    goff += real

            # ---- H. yT = (act @ w2).T * coef ----
            for hc in range(HC):
                w2_sl = sbw.tile([P, II], bf16, tag="w2sl", bufs=3)
                nc.sync.dma_start(out=w2_sl[:], in_=w2[:, hc * II:(hc + 1) * II])
                goff = 0
                for k, (nidx, real) in enumerate(_gspec):
                    gs = slice(goff, goff + real)
                    pso = psg.tile([P, real], f32, tag="mm0", name=f"pso{hc}_{k}")
                    for ic in range(IC):
                        nc.tensor.matmul(
                            out=pso[:],
                            lhsT=w2_sl[:, ic * P:(ic + 1) * P],
                            rhs=actT[ic][:, gs],
                            start=(ic == 0), stop=(ic == IC - 1),
                        )
                    yt_sb = sbw.tile([P, real], f32, tag="yt", name=f"yt{hc}_{k}")
                    nc.vector.tensor_mul(out=yt_sb[:], in0=pso[:], in1=cbc[:, gs])
                    nc.sync.dma_start(
                        out=o_yt[hc * P:(hc + 1) * P, gs], in_=yt_sb[:]
                    )
                    goff += real

    nc.compile()
    return nc


def _get_built(cap):
    if cap not in _build_cache:
        _build_cache[cap] = _build(cap)
    return _build_cache[cap]


def _make_consts(cap):
    cf = cap // 16
    tokid_np = (np.arange(NT)[None, :] * P + np.arange(P)[:, None]).astype(np.float32)
    io16_np = np.broadcast_to(
        np.tile(np.arange(16), NT)[None, :], (P, NT * 16)
    ).astype(np.float16)
    iocf_np = np.broadcast_to(
        np.tile(np.arange(cf), NT)[None, :], (P, NT * cf)
    ).astype(np.float16)
    vt16_np = np.zeros((16, P), np.float16)
    for u in range(8):
        for q in range(16):
            vt16_np[q, 16 * u + q] = 1.0
    stril_np = np.triu(np.ones((P, P), np.float32), 1)  # [k, m] = 1 if k < m
    ones128_np = np.ones((P, 1), np.float32)
    ident_np = np.eye(P, dtype=np.float32)
    return tokid_np, io16_np, iocf_np, vt16_np, stril_np, ones128_np, ident_np


def _repack_w13(w):
    # [H, II] -> [128, IC*H]: out[p, ic*H + hc*128 + i] = w[hc*128+p, ic*128+i]
    return np.ascontiguousarray(
        w.reshape(HC, P, IC, P).transpose(1, 2, 0, 3).reshape(P, IC * H)
    )


def _repack_w2(w):
    # [II, H] -> [128, HC*II]: out[p, hc*II + ic*128 + h] = w[ic*128+p, hc*128+h]
    return np.ascontiguousarray(
        w.reshape(IC, P, HC, P).transpose(1, 2, 0, 3).reshape(P, HC * II)
    )


def _run(cap, hs, gate_w, w1s, w2s, w3s, trace=False):
    import ml_dtypes
    from concourse.bass_utils import run_bass_kernel_spmd

    nc = _get_built(cap)

    bf = ml_dtypes.bfloat16
    x_hi = hs.astype(bf)
    x_lo = (hs - x_hi.astype(np.float32)).astype(bf)
    xth_np = np.ascontiguousarray(x_hi.T)
    xtl_np = np.ascontiguousarray(x_lo.T)
    gw_hi = gate_w.astype(bf)
    gw_lo = (gate_w - gw_hi.astype(np.float32)).astype(bf)
    gws_flat = np.zeros((H, 40), np.float32)
    gws_flat[:, 0:E] = gw_hi.astype(np.float32)
    gws_flat[:, 32:40] = gw_lo.astype(np.float32)
    # repack [H, 40] -> [128, HC*40]: out[p, hc*40+e] = gws[hc*128+p, e]
    gws_np = np.ascontiguousarray(
        gws_flat.reshape(HC, P, 40).transpose(1, 0, 2).reshape(P, HC * 40).astype(bf)
    )
    oh_base = np.zeros((P, NT, E), np.float32)
    x_bf = np.ascontiguousarray(x_hi)
    tokid_np, io16_np, iocf_np, vt16_np, stril_np, ones128_np, ident_np = _make_consts(cap)

    in_maps = []
    for c in range(NCORES):
        oh_c = oh_base.copy()
        oh_c[:, :, c] = 1.0
        in_maps.append({
            "xth": xth_np,
            "xtl": xtl_np,
            "gws": gws_np,
            "w1": _repack_w13(w1s[c].astype(bf)),
            "w3": _repack_w13(w3s[c].astype(bf)),
            "w2": _repack_w2(w2s[c].astype(bf)),
            "oh": oh_c.reshape(P, NT * E),
            "tokid": tokid_np,
            "tokh": tokid_np.astype(np.float16),
            "x": x_bf,
            "io16": io16_np,
            "iocf": iocf_np,
            "vt16": vt16_np,
            "stril": stril_np.astype(bf),
            "ones128": ones128_np.astype(bf),
            "ident": ident_np,
        })

    res = run_bass_kernel_spmd(nc, in_maps, list(range(NCORES)), trace=trace)
    return res


def kernel(hidden_states, gate_w, w1s, w2s, w3s, _trace=False, _cap=560):
    hs = np.ascontiguousarray(np.asarray(hidden_states, dtype=np.float32))
    gate_w = np.ascontiguousarray(np.asarray(gate_w, dtype=np.float32))
    w1s = np.asarray(w1s, dtype=np.float32)
    w2s = np.asarray(w2s, dtype=np.float32)
    w3s = np.asarray(w3s, dtype=np.float32)

    cap = _cap
    while True:
        res = _run(cap, hs, gate_w, w1s, w2s, w3s, trace=_trace)
        counts = [int(res.results[c]["o_cnt"].ravel()[0]) for c in range(NCORES)]
        if max(counts) <= cap:
            break
        # capacity overflow (won't happen for sane routing): rebuild bigger
        cap = 2048 if max(counts) > 1024 else 1024

    out = np.zeros((T, H), dtype=np.float32)
    for c in range(NCORES):
        r = res.results[c]
        cnt = counts[c]
        idx = r["o_idx"][:cnt]
        y = np.ascontiguousarray(r["o_yt"].T[:cnt])
        out[idx] += y
    kernel._last_results = res
    return out
